# revision 1
# baseline (speedup 1.0000x reference)
import numpy as np
import jax
import jax.numpy as jnp

B = 8192        # graphs
NPG = 39        # nodes per graph
N = B * NPG
NC = 8          # neuron cores
GPC = B // NC   # graphs per core
NEG = 0.2

_DIAG = np.arange(NPG)


def _gat(x, C, W, a_s, a_d, b):
    # x [G,39,fi]; C [G,39,39] with C[g,d,s] = edge count s->d (incl self loop)
    h = x @ W                                   # [G,39,fo]
    s = h @ a_s                                 # [G,39]
    d = h @ a_d                                 # [G,39]
    E = jax.nn.leaky_relu(s[:, None, :] + d[:, :, None], NEG)   # [G,d,s]
    Em = jnp.where(C > 0, E, -1e30)
    m = jnp.max(Em, axis=2, keepdims=True)
    w = C * jnp.exp(Em - m)
    denom = jnp.sum(w, axis=2, keepdims=True)
    out = (w @ h) / (denom + 1e-16) + b
    return jax.nn.relu(out)


def _fwd(x, C, params):
    (W1, as1, ad1, b1, W2, as2, ad2, b2, W3, as3, ad3, b3,
     W4, as4, ad4, b4, lw1, lb1, lw2, lb2, lw3, lb3) = params
    G = x.shape[0]
    h1 = _gat(x[..., None], C, W1, as1, ad1, b1)
    h2 = _gat(h1, C, W2, as2, ad2, b2)
    h3 = _gat(h2, C, W3, as3, ad3, b3)
    h4 = _gat(h3, C, W4, as4, ad4, b4)
    f = jnp.concatenate([
        x, h1.reshape(G, -1), h2.reshape(G, -1),
        h3.reshape(G, -1), h4.reshape(G, -1),
        jnp.max(x, axis=1, keepdims=True),
        jnp.max(h1, axis=1), jnp.max(h2, axis=1),
        jnp.max(h3, axis=1), jnp.max(h4, axis=1)], axis=1)   # [G,4560]
    f = jax.nn.relu(f @ lw1 + lb1)
    f = jax.nn.relu(f @ lw2 + lb2)
    return f @ lw3 + lb3


_pmapped = jax.pmap(_fwd, in_axes=(0, 0, None))


def _build_C(edge_index):
    src = np.asarray(edge_index[0], dtype=np.int64)
    dst = np.asarray(edge_index[1], dtype=np.int64)
    g = dst // NPG
    sl = src - g * NPG
    dl = dst - g * NPG
    idx = (g * NPG + dl) * NPG + sl
    C = np.bincount(idx, minlength=B * NPG * NPG).astype(np.float32)
    C = C.reshape(B, NPG, NPG)
    C[:, _DIAG, _DIAG] += 1.0   # self loops on every node
    return C


def kernel(**inputs):
    x = np.asarray(inputs['x'], np.float32).reshape(B, NPG)
    C = _build_C(inputs['edge_index'])
    pnames = []
    for li in range(1, 5):
        pnames += [f'W{li}', f'as{li}', f'ad{li}', f'b{li}']
    pnames += ['lw1', 'lb1', 'lw2', 'lb2', 'lw3', 'lb3']
    params = tuple(jnp.asarray(np.asarray(inputs[k], np.float32))
                   for k in pnames)
    xs = x.reshape(NC, GPC, NPG)
    Cs = C.reshape(NC, GPC, NPG, NPG)
    out = _pmapped(xs, Cs, params)
    return np.asarray(out).reshape(B, 9).astype(np.float32)


# revision 3
# speedup vs baseline: 11.7737x; 11.7737x over previous
import numpy as np
import jax
import jax.numpy as jnp

B = 8192        # graphs
NPG = 39        # nodes per graph
N = B * NPG
NC = 8          # neuron cores
GPC = B // NC   # graphs per core
NEG = 0.2

_DIAG = np.arange(NPG)


def _gat(x, C, W, a_s, a_d, b):
    # x [G,39,fi]; C [G,39,39] with C[g,d,s] = edge count s->d (incl self loop)
    G = x.shape[0]
    fi = x.shape[2]
    fo = W.shape[1]
    # single big matmul instead of batched tiny ones
    Wa = jnp.concatenate([W, (W @ a_s)[:, None], (W @ a_d)[:, None]], axis=1)
    H = x.reshape(G * NPG, fi) @ Wa                  # [G*39, fo+2]
    h = H[:, :fo].reshape(G, NPG, fo)
    s = H[:, fo].reshape(G, NPG)
    d = H[:, fo + 1].reshape(G, NPG)
    E = jax.nn.leaky_relu(s[:, None, :] + d[:, :, None], NEG)   # [G,d,s]
    Em = jnp.where(C > 0, E, -1e30)
    m = jnp.max(Em, axis=2, keepdims=True)
    w = C * jnp.exp(Em - m)                          # [G,d,s]
    denom = jnp.sum(w, axis=2, keepdims=True)
    wn = w / (denom + 1e-16)
    # per-graph contraction as 39 vectorized FMAs over all graphs
    out = jnp.zeros((G, NPG, fo), jnp.float32)
    for sn in range(NPG):
        out = out + wn[:, :, sn, None] * h[:, sn, None, :]
    return jax.nn.relu(out + b)


def _fwd(x, C, params):
    (W1, as1, ad1, b1, W2, as2, ad2, b2, W3, as3, ad3, b3,
     W4, as4, ad4, b4, lw1, lb1, lw2, lb2, lw3, lb3) = params
    G = x.shape[0]
    h1 = _gat(x[..., None], C, W1, as1, ad1, b1)
    h2 = _gat(h1, C, W2, as2, ad2, b2)
    h3 = _gat(h2, C, W3, as3, ad3, b3)
    h4 = _gat(h3, C, W4, as4, ad4, b4)
    f = jnp.concatenate([
        x, h1.reshape(G, -1), h2.reshape(G, -1),
        h3.reshape(G, -1), h4.reshape(G, -1),
        jnp.max(x, axis=1, keepdims=True),
        jnp.max(h1, axis=1), jnp.max(h2, axis=1),
        jnp.max(h3, axis=1), jnp.max(h4, axis=1)], axis=1)   # [G,4560]
    f = jax.nn.relu(f @ lw1 + lb1)
    f = jax.nn.relu(f @ lw2 + lb2)
    return f @ lw3 + lb3


_pmapped = jax.pmap(_fwd, in_axes=(0, 0, None))


def _build_C(edge_index):
    src = np.asarray(edge_index[0], dtype=np.int64)
    dst = np.asarray(edge_index[1], dtype=np.int64)
    g = dst // NPG
    sl = src - g * NPG
    dl = dst - g * NPG
    idx = (g * NPG + dl) * NPG + sl
    C = np.bincount(idx, minlength=B * NPG * NPG).astype(np.float32)
    C = C.reshape(B, NPG, NPG)
    C[:, _DIAG, _DIAG] += 1.0   # self loops on every node
    return C


def kernel(**inputs):
    x = np.asarray(inputs['x'], np.float32).reshape(B, NPG)
    C = _build_C(inputs['edge_index'])
    pnames = []
    for li in range(1, 5):
        pnames += [f'W{li}', f'as{li}', f'ad{li}', f'b{li}']
    pnames += ['lw1', 'lb1', 'lw2', 'lb2', 'lw3', 'lb3']
    params = tuple(jnp.asarray(np.asarray(inputs[k], np.float32))
                   for k in pnames)
    xs = x.reshape(NC, GPC, NPG)
    Cs = C.reshape(NC, GPC, NPG, NPG)
    out = _pmapped(xs, Cs, params)
    return np.asarray(out).reshape(B, 9).astype(np.float32)
